# revision 12
# baseline (speedup 1.0000x reference)
"""MCTC relative-position self-attention on 8 Trainium2 NeuronCores.

Sharding: core = (batch b, head-pair hp): b = core//2, heads {2*hp, 2*hp+1}
of that batch. Each core computes full attention for its 2 heads.

Skew trick: rel_pos_rotate(rel)[b,h,i,j] == D_flat[i*(L-1) + (M-1) + j]
with D = q @ E^T of shape [S, L] (L = 2M-1) — a strided DMA from a DRAM
scratch, no compute. The scratch roundtrip runs in fp8e4 (rel is only
~2.5% of the score magnitude, so fp8 quantization is harmless) because
per-core DMA is HBM-bound (~380GB/s serial) and the roundtrip is the
biggest traffic term.

All matmuls run in fp16 (full PE rate: 1 cycle/row vs 4 for fp32) with
fp32 PSUM accumulation. X^T is pre-transposed on the host (free), the
softmax normalization is done on the host from the exp row-sums
(activation accum_out), and the PSUM->SBUF copies are spread across
DVE / Act so the PE stays the bottleneck. Head-0's q projection runs
kd-outer across 6 PSUM banks so the PE pipelines against the arriving
per-kd input DMAs instead of waiting for the full load.
"""

import math
import sys

if "/opt/trn_rl_repo" not in sys.path:
    sys.path.insert(0, "/opt/trn_rl_repo")

import numpy as np

import concourse.bass as bass
import concourse.mybir as mybir
import concourse.tile as tile
from concourse import bacc
from concourse.bass_utils import run_bass_kernel_spmd
from concourse.masks import make_identity

S = 920
DMODEL = 1536
HD = 384
M = 920
L = 2 * M - 1  # 1839
NH_PER_CORE = 2
WH = NH_PER_CORE * HD  # 768 weight columns per core

F32 = mybir.dt.float32
F16 = mybir.dt.float16
F8 = mybir.dt.float8e4

P = 128
NS = 8  # ceil(920/128) s-chunks, last has 24 rows
ND = 12  # 1536/128 contraction chunks for projections
NF = 3  # 384/128 feature chunks
NQK = 460  # half of 920, fits one PSUM bank
DW = 1056  # padded D-chunk width (>= 919+128)


def _pc(c):
    return min(P, S - c * P)


def build_kernel():
    nc = bacc.Bacc("TRN2", target_bir_lowering=False, debug=False)

    xt_d = nc.dram_tensor("xt", [DMODEL, S], F16, kind="ExternalInput")
    wq_d = nc.dram_tensor("wq", [DMODEL, WH], F16, kind="ExternalInput")
    wk_d = nc.dram_tensor("wk", [DMODEL, WH], F16, kind="ExternalInput")
    wv_d = nc.dram_tensor("wv", [DMODEL, WH], F16, kind="ExternalInput")
    et_d = nc.dram_tensor("et", [HD, L], F16, kind="ExternalInput")
    out_d = nc.dram_tensor("out", [NH_PER_CORE, S, HD], F16, kind="ExternalOutput")
    den_d = nc.dram_tensor("den", [NH_PER_CORE, P, NS, 2], F32, kind="ExternalOutput")

    from contextlib import ExitStack

    with tile.TileContext(nc) as tc, ExitStack() as ctx:
        ep = ctx.enter_context
        small_pool = ep(tc.tile_pool(name="small", bufs=1))
        xt_pool = ep(tc.tile_pool(name="xt", bufs=1))
        w_pool = ep(tc.tile_pool(name="w", bufs=1))
        et_pool = ep(tc.tile_pool(name="et", bufs=1))
        qkt_pool = ep(tc.tile_pool(name="qkt", bufs=2))
        v_pool = ep(tc.tile_pool(name="vsb", bufs=1))
        p_pool = ep(tc.tile_pool(name="psb", bufs=1))
        pT_pool = ep(tc.tile_pool(name="pT", bufs=1))
        rel_pool = ep(tc.tile_pool(name="rel", bufs=2))
        dst_pool = ep(tc.tile_pool(name="dstage", bufs=3))
        o_pool = ep(tc.tile_pool(name="outp", bufs=2))
        den_pool = ep(tc.tile_pool(name="den", bufs=2))
        pmm = ep(tc.tile_pool(name="pmm", bufs=6, space="PSUM"))
        pt = ep(tc.tile_pool(name="pt", bufs=2, space="PSUM"))
        dram_pool = ep(tc.tile_pool(name="dram", bufs=2, space="DRAM"))

        ident = small_pool.tile([P, P], F16, tag="ident")
        make_identity(nc, ident)

        # ---- per-kd input tiles, interleaved DMAs: compute starts early ----
        xt_t = [xt_pool.tile([P, S], F16, tag=f"xt{kd}", name=f"xt{kd}")
                for kd in range(ND)]
        wq_t = [w_pool.tile([P, WH], F16, tag=f"wq{kd}", name=f"wq{kd}")
                for kd in range(ND)]
        wk_t = [w_pool.tile([P, WH], F16, tag=f"wk{kd}", name=f"wk{kd}")
                for kd in range(ND)]
        wv_t = [w_pool.tile([P, WH], F16, tag=f"wv{kd}", name=f"wv{kd}")
                for kd in range(ND)]
        xt_view = xt_d.ap().rearrange("(kd p) s -> p kd s", p=P)
        wq_view = wq_d.ap().rearrange("(kd p) f -> p kd f", p=P)
        wk_view = wk_d.ap().rearrange("(kd p) f -> p kd f", p=P)
        wv_view = wv_d.ap().rearrange("(kd p) f -> p kd f", p=P)
        for kd in range(ND):
            nc.sync.dma_start(xt_t[kd][:], xt_view[:, kd, :])
            nc.sync.dma_start(wq_t[kd][:], wq_view[:, kd, :])
        for kd in range(ND):
            nc.sync.dma_start(wk_t[kd][:], wk_view[:, kd, :])

        et_sb = et_pool.tile([P, NF, L], F16, tag="et")
        et_view = et_d.ap().rearrange("(j p) l -> p j l", p=P)
        for j in range(NF):
            nc.sync.dma_start(et_sb[:, j, :], et_view[:, j, :])
        for kd in range(ND):
            nc.sync.dma_start(wv_t[kd][:], wv_view[:, kd, :])

        for h in range(NH_PER_CORE):
            hs = h * HD

            # ---- q^T / k^T: [384, 920] ----
            qT_sb = qkt_pool.tile([P, NF, S], F16, tag="qT")
            kT_sb = qkt_pool.tile([P, NF, S], F16, tag="kT")
            if h == 0:
                # kd-outer over 6 PSUM banks: pipeline against input DMAs
                ps6 = [pmm.tile([P, NQK], F32, tag="pmm", name=f"ps{i}")
                       for i in range(6)]
                for kd in range(ND):
                    for m in range(NF):
                        wch = wq_t[kd][:, hs + m * P : hs + (m + 1) * P]
                        for n in range(2):
                            nc.tensor.matmul(
                                ps6[2 * m + n][:], wch,
                                xt_t[kd][:, n * NQK : (n + 1) * NQK],
                                start=(kd == 0), stop=(kd == ND - 1),
                            )
                for m in range(NF):
                    for n in range(2):
                        nc.vector.tensor_copy(
                            qT_sb[:, m, n * NQK : (n + 1) * NQK], ps6[2 * m + n][:]
                        )
                proj = ((wk_t, kT_sb),)
            else:
                proj = ((wq_t, qT_sb), (wk_t, kT_sb))
            for w_t, dst in proj:
                for m in range(NF):
                    ps0 = pmm.tile([P, NQK], F32, tag="pmm")
                    ps1 = pmm.tile([P, NQK], F32, tag="pmm")
                    for kd in range(ND):
                        wch = w_t[kd][:, hs + m * P : hs + (m + 1) * P]
                        nc.tensor.matmul(
                            ps0[:], wch, xt_t[kd][:, :NQK],
                            start=(kd == 0), stop=(kd == ND - 1),
                        )
                        nc.tensor.matmul(
                            ps1[:], wch, xt_t[kd][:, NQK:],
                            start=(kd == 0), stop=(kd == ND - 1),
                        )
                    nc.vector.tensor_copy(dst[:, m, :NQK], ps0[:])
                    nc.vector.tensor_copy(dst[:, m, NQK:], ps1[:])

            # ---- D = q E^T -> fp8 DRAM scratch; prefetch skewed rel rows ----
            d_dram = dram_pool.tile([S, L], F8, tag="dscratch")
            d_flat = d_dram.rearrange("a b -> (a b)")
            rel_all = rel_pool.tile([P, NS, S], F8, tag="rel")
            for c in range(NS):
                pc = _pc(c)
                i_max = c * P + pc - 1
                l_lo = (M - 1) - i_max
                l_hi = (L - 1) - c * P + 1
                width = l_hi - l_lo
                nt = 3
                base = width // nt
                sizes = [base + (1 if i < width % nt else 0) for i in range(nt)]
                dstg = dst_pool.tile([P, DW], F8, tag="dstg")
                off = 0
                for si, w in enumerate(sizes):
                    ps = pmm.tile([P, NQK], F32, tag="pmm")
                    for kd in range(NF):
                        nc.tensor.matmul(
                            ps[:pc, :w],
                            qT_sb[:, kd, c * P : c * P + pc],
                            et_sb[:, kd, l_lo + off : l_lo + off + w],
                            start=(kd == 0), stop=(kd == NF - 1),
                        )
                    if si == 1:
                        nc.scalar.copy(dstg[:pc, off : off + w], ps[:pc, :w])
                    else:
                        nc.vector.tensor_copy(dstg[:pc, off : off + w], ps[:pc, :w])
                    off += w
                nc.sync.dma_start(
                    d_dram[c * P : c * P + pc, l_lo : l_lo + width],
                    dstg[:pc, :width],
                )
                skew = (
                    d_flat[
                        (M - 1) + c * P * (L - 1) :
                        (M - 1) + c * P * (L - 1) + pc * (L - 1)
                    ]
                    .rearrange("(p x) -> p x", x=L - 1)
                )
                nc.sync.dma_start(rel_all[:pc, c, :], skew[:, :S])

            # ---- v projection (natural layout): [920, 384] ----
            v_sb = v_pool.tile([P, NS, HD], F16, tag="v")
            for c in range(NS):
                pc = _pc(c)
                ps = pmm.tile([P, HD], F32, tag="pmm")
                for kd in range(ND):
                    nc.tensor.matmul(
                        ps[:pc, :], xt_t[kd][:, c * P : c * P + pc],
                        wv_t[kd][:, hs : hs + HD],
                        start=(kd == 0), stop=(kd == ND - 1),
                    )
                nc.vector.tensor_copy(v_sb[:pc, c, :], ps[:pc, :])

            # ---- scores = qk + rel, exp (+half row-sums) ----
            den_sb = den_pool.tile([P, NS, 2], F32, tag="den")
            p_sb = p_pool.tile([P, NS, S], F16, tag="p")
            for c in range(NS):
                pc = _pc(c)
                for n in range(2):
                    ps = pmm.tile([P, NQK], F32, tag="pmm")
                    for kd in range(NF):
                        nc.tensor.matmul(
                            ps[:pc, :],
                            qT_sb[:, kd, c * P : c * P + pc],
                            kT_sb[:, kd, n * NQK : (n + 1) * NQK],
                            start=(kd == 0), stop=(kd == NF - 1),
                        )
                    nc.vector.tensor_add(
                        ps[:pc, :], ps[:pc, :],
                        rel_all[:pc, c, n * NQK : (n + 1) * NQK],
                    )
                    nc.scalar.activation(
                        p_sb[:pc, c, n * NQK : (n + 1) * NQK],
                        ps[:pc, :],
                        mybir.ActivationFunctionType.Exp,
                        scale=float(1.0 / math.sqrt(HD)),
                        accum_out=den_sb[:pc, c, n : n + 1],
                    )
            nc.sync.dma_start(den_d.ap()[h], den_sb[:])

            # ---- transpose probs: per kc, 8 blocks into one fp16 bank ----
            pT_sb = pT_pool.tile([P, NS, S], F16, tag="pT")
            for kc in range(NS):
                pkc = _pc(kc)
                ptile = pt.tile([P, S], F16, tag="pt")
                for c in range(NS):
                    pcc = _pc(c)
                    nc.tensor.transpose(
                        ptile[:pkc, c * P : c * P + pcc],
                        p_sb[:pcc, c, kc * P : kc * P + pkc],
                        ident[:pcc, :pcc],
                    )
                if kc % 2 == 0:
                    nc.vector.tensor_copy(pT_sb[:pkc, kc, :], ptile[:pkc, :])
                else:
                    nc.scalar.copy(pT_sb[:pkc, kc, :], ptile[:pkc, :])

            # ---- ctx_unnorm = P^T.T @ v  (normalized on host) ----
            for c in range(NS):
                pc = _pc(c)
                ps = pmm.tile([P, HD], F32, tag="pmm")
                for kc in range(NS):
                    pkc = _pc(kc)
                    nc.tensor.matmul(
                        ps[:pc, :],
                        pT_sb[:pkc, kc, c * P : c * P + pc],
                        v_sb[:pkc, kc, :],
                        start=(kc == 0), stop=(kc == NS - 1),
                    )
                o_sb = o_pool.tile([P, HD], F16, tag="o")
                nc.scalar.copy(o_sb[:pc, :], ps[:pc, :])
                nc.sync.dma_start(
                    out_d.ap()[h, c * P : c * P + pc, :], o_sb[:pc, :]
                )

    nc.compile()
    return nc


_NC = None
LAST_RESULTS = None


def kernel(hidden_states, q_w, k_w, v_w, dist_emb):
    global _NC, LAST_RESULTS
    if _NC is None:
        _NC = build_kernel()

    hidden_states = np.asarray(hidden_states, dtype=np.float32)
    q_w = np.asarray(q_w, dtype=np.float32)
    k_w = np.asarray(k_w, dtype=np.float32)
    v_w = np.asarray(v_w, dtype=np.float32)
    dist_emb = np.asarray(dist_emb, dtype=np.float32)

    B = hidden_states.shape[0]
    et = np.ascontiguousarray(dist_emb.T.astype(np.float16))
    xts = [
        np.ascontiguousarray(hidden_states[b].T.astype(np.float16))
        for b in range(B)
    ]
    in_maps = []
    for core in range(8):
        b, hp = core // 2, core % 2
        sl = slice(hp * WH, (hp + 1) * WH)
        in_maps.append(
            {
                "xt": xts[b],
                "wq": np.ascontiguousarray(q_w[:, sl].astype(np.float16)),
                "wk": np.ascontiguousarray(k_w[:, sl].astype(np.float16)),
                "wv": np.ascontiguousarray(v_w[:, sl].astype(np.float16)),
                "et": et,
            }
        )

    res = run_bass_kernel_spmd(_NC, in_maps, core_ids=list(range(8)))
    LAST_RESULTS = res

    out = np.empty((B, S, 4 * HD), np.float32)
    for core in range(8):
        b, hp = core // 2, core % 2
        o = res.results[core]["out"]  # [2, S, HD] fp16, unnormalized
        den = res.results[core]["den"]  # [2, P, NS, 2] fp32 half row-sums
        for j in range(NH_PER_CORE):
            h = hp * NH_PER_CORE + j
            dh = den[j].sum(-1)  # [P, NS]
            denom_rows = dh.T.reshape(-1)[:S]  # row i = c*128+p -> dh[p, c]
            out[b, :, h * HD : (h + 1) * HD] = (
                o[j].astype(np.float32) / denom_rows[:, None]
            )
    return out


# revision 13
# speedup vs baseline: 1.0018x; 1.0018x over previous
"""MCTC relative-position self-attention on 8 Trainium2 NeuronCores.

Sharding: core = (batch b, head-pair hp): b = core//2, heads {2*hp, 2*hp+1}
of that batch. Each core computes full attention for its 2 heads.

Skew trick: rel_pos_rotate(rel)[b,h,i,j] == D_flat[i*(L-1) + (M-1) + j]
with D = q @ E^T of shape [S, L] (L = 2M-1) — a strided DMA from a DRAM
scratch, no compute. The scratch roundtrip runs in fp8e4 (rel is only
~2.5% of the score magnitude, so fp8 quantization is harmless) because
per-core DMA is HBM-bound (~380GB/s serial) and the roundtrip is the
biggest traffic term.

All matmuls run in fp16 (full PE rate: 1 cycle/row vs 4 for fp32) with
fp32 PSUM accumulation. X^T is pre-transposed on the host (free), the
softmax normalization is done on the host from the exp row-sums
(activation accum_out), and the PSUM->SBUF copies are spread across
DVE / Act so the PE stays the bottleneck. Head-0's q projection runs
kd-outer across 6 PSUM banks so the PE pipelines against the arriving
per-kd input DMAs instead of waiting for the full load.
"""

import math
import sys

if "/opt/trn_rl_repo" not in sys.path:
    sys.path.insert(0, "/opt/trn_rl_repo")

import numpy as np

import concourse.bass as bass
import concourse.mybir as mybir
import concourse.tile as tile
from concourse import bacc
from concourse.bass_utils import run_bass_kernel_spmd
from concourse.masks import make_identity

S = 920
DMODEL = 1536
HD = 384
M = 920
L = 2 * M - 1  # 1839
NH_PER_CORE = 2
WH = NH_PER_CORE * HD  # 768 weight columns per core

F32 = mybir.dt.float32
F16 = mybir.dt.float16
F8 = mybir.dt.float8e4

P = 128
NS = 8  # ceil(920/128) s-chunks, last has 24 rows
ND = 12  # 1536/128 contraction chunks for projections
NF = 3  # 384/128 feature chunks
NQK = 460  # half of 920, fits one PSUM bank
DW = 1056  # padded D-chunk width (>= 919+128)


def _pc(c):
    return min(P, S - c * P)


def build_kernel():
    nc = bacc.Bacc("TRN2", target_bir_lowering=False, debug=False)

    xt_d = nc.dram_tensor("xt", [DMODEL, S], F16, kind="ExternalInput")
    wq_d = nc.dram_tensor("wq", [DMODEL, WH], F16, kind="ExternalInput")
    wk_d = nc.dram_tensor("wk", [DMODEL, WH], F16, kind="ExternalInput")
    wv_d = nc.dram_tensor("wv", [DMODEL, WH], F16, kind="ExternalInput")
    # E^T padded to 512 rows, fp8, 4 slots of 128 (DoubleRow pairs)
    et_d = nc.dram_tensor("et", [4 * P, L], F8, kind="ExternalInput")
    out_d = nc.dram_tensor("out", [NH_PER_CORE, S, HD], F16, kind="ExternalOutput")
    den_d = nc.dram_tensor("den", [NH_PER_CORE, P, NS, 2], F32, kind="ExternalOutput")

    from contextlib import ExitStack

    with tile.TileContext(nc) as tc, ExitStack() as ctx:
        ep = ctx.enter_context
        small_pool = ep(tc.tile_pool(name="small", bufs=1))
        xt_pool = ep(tc.tile_pool(name="xt", bufs=1))
        w_pool = ep(tc.tile_pool(name="w", bufs=1))
        et_pool = ep(tc.tile_pool(name="et", bufs=1))
        qkt_pool = ep(tc.tile_pool(name="qkt", bufs=2))
        v_pool = ep(tc.tile_pool(name="vsb", bufs=1))
        p_pool = ep(tc.tile_pool(name="psb", bufs=1))
        pT_pool = ep(tc.tile_pool(name="pT", bufs=1))
        rel_pool = ep(tc.tile_pool(name="rel", bufs=2))
        dst_pool = ep(tc.tile_pool(name="dstage", bufs=3))
        o_pool = ep(tc.tile_pool(name="outp", bufs=2))
        den_pool = ep(tc.tile_pool(name="den", bufs=2))
        pmm = ep(tc.tile_pool(name="pmm", bufs=6, space="PSUM"))
        pt = ep(tc.tile_pool(name="pt", bufs=2, space="PSUM"))
        dram_pool = ep(tc.tile_pool(name="dram", bufs=2, space="DRAM"))

        ident = small_pool.tile([P, P], F16, tag="ident")
        make_identity(nc, ident)

        # ---- per-kd input tiles, interleaved DMAs: compute starts early ----
        xt_t = [xt_pool.tile([P, S], F16, tag=f"xt{kd}", name=f"xt{kd}")
                for kd in range(ND)]
        wq_t = [w_pool.tile([P, WH], F16, tag=f"wq{kd}", name=f"wq{kd}")
                for kd in range(ND)]
        wk_t = [w_pool.tile([P, WH], F16, tag=f"wk{kd}", name=f"wk{kd}")
                for kd in range(ND)]
        wv_t = [w_pool.tile([P, WH], F16, tag=f"wv{kd}", name=f"wv{kd}")
                for kd in range(ND)]
        xt_view = xt_d.ap().rearrange("(kd p) s -> p kd s", p=P)
        wq_view = wq_d.ap().rearrange("(kd p) f -> p kd f", p=P)
        wk_view = wk_d.ap().rearrange("(kd p) f -> p kd f", p=P)
        wv_view = wv_d.ap().rearrange("(kd p) f -> p kd f", p=P)
        for kd in range(ND):
            nc.sync.dma_start(xt_t[kd][:], xt_view[:, kd, :])
            nc.sync.dma_start(wq_t[kd][:], wq_view[:, kd, :])
        for kd in range(ND):
            nc.sync.dma_start(wk_t[kd][:], wk_view[:, kd, :])

        et_sb = et_pool.tile([P, 4, L], F8, tag="et")
        et_view = et_d.ap().rearrange("(j p) l -> p j l", p=P)
        for j in range(4):
            nc.sync.dma_start(et_sb[:, j, :], et_view[:, j, :])
        for kd in range(ND):
            nc.sync.dma_start(wv_t[kd][:], wv_view[:, kd, :])

        for h in range(NH_PER_CORE):
            hs = h * HD

            # ---- q^T / k^T: [384, 920] ----
            qT_sb = qkt_pool.tile([P, NF, S], F16, tag="qT")
            kT_sb = qkt_pool.tile([P, NF, S], F16, tag="kT")
            if h == 0:
                # kd-outer over 6 PSUM banks: pipeline against input DMAs
                ps6 = [pmm.tile([P, NQK], F32, tag="pmm", name=f"ps{i}")
                       for i in range(6)]
                for kd in range(ND):
                    for m in range(NF):
                        wch = wq_t[kd][:, hs + m * P : hs + (m + 1) * P]
                        for n in range(2):
                            nc.tensor.matmul(
                                ps6[2 * m + n][:], wch,
                                xt_t[kd][:, n * NQK : (n + 1) * NQK],
                                start=(kd == 0), stop=(kd == ND - 1),
                            )
                for m in range(NF):
                    for n in range(2):
                        nc.vector.tensor_copy(
                            qT_sb[:, m, n * NQK : (n + 1) * NQK], ps6[2 * m + n][:]
                        )
                proj = ((wk_t, kT_sb),)
            else:
                proj = ((wq_t, qT_sb), (wk_t, kT_sb))
            for w_t, dst in proj:
                for m in range(NF):
                    ps0 = pmm.tile([P, NQK], F32, tag="pmm")
                    ps1 = pmm.tile([P, NQK], F32, tag="pmm")
                    for kd in range(ND):
                        wch = w_t[kd][:, hs + m * P : hs + (m + 1) * P]
                        nc.tensor.matmul(
                            ps0[:], wch, xt_t[kd][:, :NQK],
                            start=(kd == 0), stop=(kd == ND - 1),
                        )
                        nc.tensor.matmul(
                            ps1[:], wch, xt_t[kd][:, NQK:],
                            start=(kd == 0), stop=(kd == ND - 1),
                        )
                    nc.vector.tensor_copy(dst[:, m, :NQK], ps0[:])
                    nc.vector.tensor_copy(dst[:, m, NQK:], ps1[:])

            # ---- qT chunk-blocked 4-slot fp8 for DoubleRow (pair-contiguous;
            # the dual-fp8 LDWEIGHTS ISA check requires slot stride == M) ----
            qT8 = qkt_pool.tile([P, NS, 2, 2, P], F8, tag="qT8")
            nc.gpsimd.memset(qT8[:], 0.0)
            for c8 in range(NS):
                pc8 = _pc(c8)
                for blk in range(NF):
                    nc.gpsimd.tensor_copy(
                        qT8[:, c8, blk // 2, blk % 2, :pc8],
                        qT_sb[:, blk, c8 * P : c8 * P + pc8],
                    )

            # ---- D = q E^T -> fp8 DRAM scratch; prefetch skewed rel rows ----
            d_dram = dram_pool.tile([S, L], F8, tag="dscratch")
            d_flat = d_dram.rearrange("a b -> (a b)")
            rel_all = rel_pool.tile([P, NS, S], F8, tag="rel")
            for c in range(NS):
                pc = _pc(c)
                i_max = c * P + pc - 1
                l_lo = (M - 1) - i_max
                l_hi = (L - 1) - c * P + 1
                width = l_hi - l_lo
                nt = 3
                base = width // nt
                sizes = [base + (1 if i < width % nt else 0) for i in range(nt)]
                dstg = dst_pool.tile([P, DW], F8, tag="dstg")
                off = 0
                for si, w in enumerate(sizes):
                    ps = pmm.tile([P, NQK], F32, tag="pmm")
                    for g in range(2):
                        nc.tensor.matmul(
                            ps[:pc, :w],
                            qT8[:, c, g, :, :pc],
                            et_sb[:, 2 * g : 2 * g + 2,
                                  l_lo + off : l_lo + off + w],
                            start=(g == 0), stop=(g == 1),
                            perf_mode=mybir.MatmulPerfMode.DoubleRow,
                        )
                    if si == 1:
                        nc.scalar.copy(dstg[:pc, off : off + w], ps[:pc, :w])
                    else:
                        nc.vector.tensor_copy(dstg[:pc, off : off + w], ps[:pc, :w])
                    off += w
                nc.sync.dma_start(
                    d_dram[c * P : c * P + pc, l_lo : l_lo + width],
                    dstg[:pc, :width],
                )
                skew = (
                    d_flat[
                        (M - 1) + c * P * (L - 1) :
                        (M - 1) + c * P * (L - 1) + pc * (L - 1)
                    ]
                    .rearrange("(p x) -> p x", x=L - 1)
                )
                nc.sync.dma_start(rel_all[:pc, c, :], skew[:, :S])

            # ---- v projection (natural layout): [920, 384] ----
            v_sb = v_pool.tile([P, NS, HD], F16, tag="v")
            for c in range(NS):
                pc = _pc(c)
                ps = pmm.tile([P, HD], F32, tag="pmm")
                for kd in range(ND):
                    nc.tensor.matmul(
                        ps[:pc, :], xt_t[kd][:, c * P : c * P + pc],
                        wv_t[kd][:, hs : hs + HD],
                        start=(kd == 0), stop=(kd == ND - 1),
                    )
                nc.vector.tensor_copy(v_sb[:pc, c, :], ps[:pc, :])

            # ---- scores = qk + rel, exp (+half row-sums) ----
            den_sb = den_pool.tile([P, NS, 2], F32, tag="den")
            p_sb = p_pool.tile([P, NS, S], F16, tag="p")
            for c in range(NS):
                pc = _pc(c)
                for n in range(2):
                    ps = pmm.tile([P, NQK], F32, tag="pmm")
                    for kd in range(NF):
                        nc.tensor.matmul(
                            ps[:pc, :],
                            qT_sb[:, kd, c * P : c * P + pc],
                            kT_sb[:, kd, n * NQK : (n + 1) * NQK],
                            start=(kd == 0), stop=(kd == NF - 1),
                        )
                    nc.vector.tensor_add(
                        ps[:pc, :], ps[:pc, :],
                        rel_all[:pc, c, n * NQK : (n + 1) * NQK],
                    )
                    nc.scalar.activation(
                        p_sb[:pc, c, n * NQK : (n + 1) * NQK],
                        ps[:pc, :],
                        mybir.ActivationFunctionType.Exp,
                        scale=float(1.0 / math.sqrt(HD)),
                        accum_out=den_sb[:pc, c, n : n + 1],
                    )
            nc.sync.dma_start(den_d.ap()[h], den_sb[:])

            # ---- transpose probs: per kc, 8 blocks into one fp16 bank ----
            pT_sb = pT_pool.tile([P, NS, S], F16, tag="pT")
            for kc in range(NS):
                pkc = _pc(kc)
                ptile = pt.tile([P, S], F16, tag="pt")
                for c in range(NS):
                    pcc = _pc(c)
                    nc.tensor.transpose(
                        ptile[:pkc, c * P : c * P + pcc],
                        p_sb[:pcc, c, kc * P : kc * P + pkc],
                        ident[:pcc, :pcc],
                    )
                if kc % 2 == 0:
                    nc.vector.tensor_copy(pT_sb[:pkc, kc, :], ptile[:pkc, :])
                else:
                    nc.scalar.copy(pT_sb[:pkc, kc, :], ptile[:pkc, :])

            # ---- ctx_unnorm = P^T.T @ v  (normalized on host) ----
            for c in range(NS):
                pc = _pc(c)
                ps = pmm.tile([P, HD], F32, tag="pmm")
                for kc in range(NS):
                    pkc = _pc(kc)
                    nc.tensor.matmul(
                        ps[:pc, :],
                        pT_sb[:pkc, kc, c * P : c * P + pc],
                        v_sb[:pkc, kc, :],
                        start=(kc == 0), stop=(kc == NS - 1),
                    )
                o_sb = o_pool.tile([P, HD], F16, tag="o")
                nc.scalar.copy(o_sb[:pc, :], ps[:pc, :])
                nc.sync.dma_start(
                    out_d.ap()[h, c * P : c * P + pc, :], o_sb[:pc, :]
                )

    nc.compile()
    return nc


_NC = None
LAST_RESULTS = None


def kernel(hidden_states, q_w, k_w, v_w, dist_emb):
    global _NC, LAST_RESULTS
    if _NC is None:
        _NC = build_kernel()

    hidden_states = np.asarray(hidden_states, dtype=np.float32)
    q_w = np.asarray(q_w, dtype=np.float32)
    k_w = np.asarray(k_w, dtype=np.float32)
    v_w = np.asarray(v_w, dtype=np.float32)
    dist_emb = np.asarray(dist_emb, dtype=np.float32)

    import ml_dtypes

    B = hidden_states.shape[0]
    et = np.zeros((4 * P, L), dtype=ml_dtypes.float8_e4m3)
    et[:HD] = dist_emb.T.astype(ml_dtypes.float8_e4m3)
    xts = [
        np.ascontiguousarray(hidden_states[b].T.astype(np.float16))
        for b in range(B)
    ]
    in_maps = []
    for core in range(8):
        b, hp = core // 2, core % 2
        sl = slice(hp * WH, (hp + 1) * WH)
        in_maps.append(
            {
                "xt": xts[b],
                "wq": np.ascontiguousarray(q_w[:, sl].astype(np.float16)),
                "wk": np.ascontiguousarray(k_w[:, sl].astype(np.float16)),
                "wv": np.ascontiguousarray(v_w[:, sl].astype(np.float16)),
                "et": et,
            }
        )

    res = run_bass_kernel_spmd(_NC, in_maps, core_ids=list(range(8)))
    LAST_RESULTS = res

    out = np.empty((B, S, 4 * HD), np.float32)
    for core in range(8):
        b, hp = core // 2, core % 2
        o = res.results[core]["out"]  # [2, S, HD] fp16, unnormalized
        den = res.results[core]["den"]  # [2, P, NS, 2] fp32 half row-sums
        for j in range(NH_PER_CORE):
            h = hp * NH_PER_CORE + j
            dh = den[j].sum(-1)  # [P, NS]
            denom_rows = dh.T.reshape(-1)[:S]  # row i = c*128+p -> dh[p, c]
            out[b, :, h * HD : (h + 1) * HD] = (
                o[j].astype(np.float32) / denom_rows[:, None]
            )
    return out
